# revision 3
# baseline (speedup 1.0000x reference)
"""Trainium2 Bass kernel for nn_CelltypeScaleLayer (segment gather + scale + transpose).

Reference computation:
    z = x[idx.reshape(-1)] * repeat(weight, M)[:, None]   # (NJ, NCELL)
    out = z.T.reshape(-1)                                 # (NCELL * NJ,)

Sharding: data-parallel over the NJ (gathered-row) axis. Core m owns output
columns j in [m*JPC, (m+1)*JPC) of the (NCELL, NJ) output, i.e. a contiguous
slab of the flattened output. x is replicated in bf16.

Per-core pipeline, per chunk of CHUNK=896 j positions:
  1. SWDGE dma_gather with transpose=True: rows x_bf16[idx[j], :] (1KB each)
     from HBM land TRANSPOSED in SBUF: partition = cell%128, free = (cell//128, j).
     Data arrives directly in output layout - no PE transpose, no PSUM.
  2. One DVE tensor_mul against a host-prepared broadcast weight tile
     w_sb[p, j] = w[j] (= weight[j//M]), using a 0-stride AP over the 4
     cell-blocks.
  3. One HWDGE DMA per chunk: SBUF (128, 4, chunk) -> HBM out rows, DRAM side
     viewed as (p, blk, j) so out row c = blk*128 + p.

Everything is bf16 on device (harness gate is rel_err < 2e-2; measured
rel err 1.07e-2 from bf16 rounding of x, w, and the product). Host converts
the bf16 output back to f32.

DMA traffic per core: 12.8 MB gather read + 12.8 MB write. The two streams
overlap on HW; measured repeat-delta exec 36-68 us/iter (contention-noisy,
best 36.1 us ~= the 12.8MB/360GB/s single-stream roofline) vs 144.7 us f32
baseline. TimelineSim (which serializes all DMA at 360 GB/s) predicts
71.3 us/iter steady-state.
"""

import numpy as np
import ml_dtypes

import concourse.bacc as bacc
import concourse.tile as tile
import concourse.mybir as mybir
from concourse.bass import broadcast_tensor_aps
from concourse.bass_utils import run_bass_kernel_spmd

F32 = mybir.dt.float32
BF16 = mybir.dt.bfloat16
I16 = mybir.dt.int16
BF16NP = np.dtype(ml_dtypes.bfloat16)

# Problem shape (hardcoded per the harness contract).
NF = 20000        # x rows (features)
NCELL = 512       # x cols (cells) == output rows
NCT = 50          # celltypes
M = 2000          # rows gathered per celltype
NJ = NCT * M      # 100000 gathered rows == output cols
NBLK = NCELL // 128  # 4 cell blocks (partition = cell%128, blk = cell//128)

NCORES = 8
JPC = NJ // NCORES          # 12500 output columns per core
CHUNK = 896                 # gather indices per dma_gather call (7 groups of 128).
                            # >=1280 per SWDGE gather crashes the device
                            # (descriptor-ring capacity is 1024 entries).
NCHUNK = -(-JPC // CHUNK)   # 14
NIDX = NCHUNK * CHUNK       # 12544 (tail padded with idx 0, cols never written)

_cached = None


def _build(repeats=1, ncores=NCORES):
    """Build + compile the SPMD program. `repeats` re-runs the whole pipeline
    that many times inside one NEFF (used only for timing measurements)."""
    nc = bacc.Bacc("TRN2", target_bir_lowering=False, debug=False,
                   num_devices=ncores)
    x = nc.dram_tensor("x", [NF, NCELL], BF16, kind="ExternalInput")
    idxs = nc.dram_tensor("idxs", [128, NIDX // 16], I16, kind="ExternalInput")
    wrow = nc.dram_tensor("wrow", [128, NIDX], BF16, kind="ExternalInput")
    out = nc.dram_tensor("out", [NCELL, JPC], BF16, kind="ExternalOutput")

    # DRAM view matching the gather's transposed SBUF layout: cell c lives at
    # (partition c%128, blk c//128), i.e. out row c = blk*128 + p.
    out_v = out.ap().rearrange("(b p) j -> p b j", p=128)

    with tile.TileContext(nc) as tc:
        with tc.tile_pool(name="const", bufs=1) as cpool:
            idx_sb = cpool.tile([128, NIDX // 16], I16)
            nc.sync.dma_start(idx_sb[:], idxs.ap())
            w_sb = cpool.tile([128, NIDX], BF16)
            nc.sync.dma_start(w_sb[:], wrow.ap())

            with tc.tile_pool(name="gpool", bufs=6) as gpool:
                for _ in range(repeats):
                    for k in range(NCHUNK):
                        nvalid = min(JPC - k * CHUNK, CHUNK)
                        gb = gpool.tile([128, NBLK, CHUNK], BF16, tag="gb")
                        nc.gpsimd.dma_gather(
                            gb[:],
                            x.ap(),
                            idx_sb[:, k * (CHUNK // 16):(k + 1) * (CHUNK // 16)],
                            CHUNK,
                            CHUNK,
                            NCELL,
                            transpose=True,
                        )
                        w3 = w_sb[:, k * CHUNK:(k + 1) * CHUNK].rearrange(
                            "p (o j) -> p o j", o=1)
                        aa, ww = broadcast_tensor_aps(gb[:, :, :], w3)
                        nc.vector.tensor_mul(gb[:, :, :], aa, ww)
                        nc.sync.dma_start(
                            out_v[:, :, k * CHUNK:k * CHUNK + nvalid],
                            gb[:, :, :nvalid],
                        )
    nc.compile()
    return nc


def _host_prep(x, weight, idx, ncores=NCORES):
    x = np.asarray(x, dtype=np.float32)
    xbf = np.ascontiguousarray(x.astype(BF16NP))
    weight = np.asarray(weight, dtype=np.float32)
    idx_flat = np.asarray(idx).reshape(-1).astype(np.int64)
    w_exp = np.repeat(weight, M).astype(np.float32)  # (NJ,) per-j weight

    in_maps = []
    for m in range(ncores):
        j0 = m * JPC
        padded = np.zeros((NIDX,), dtype=np.int64)
        padded[:JPC] = idx_flat[j0:j0 + JPC]
        # dma_gather index layout: index i lives at partition i%16, free i//16,
        # replicated across the 8 Q7 core groups.
        wrapped16 = padded.reshape(NIDX // 16, 16).T.astype(np.int16)
        wrapped = np.ascontiguousarray(np.tile(wrapped16, (8, 1)))  # (128, NIDX//16)

        # broadcast weight row: wrow[p, j] = w for global j = j0 + j, all p
        wpad = np.ones((NIDX,), dtype=np.float32)
        wpad[:JPC] = w_exp[j0:j0 + JPC]
        wtile = np.ascontiguousarray(
            np.broadcast_to(wpad.astype(BF16NP), (128, NIDX)))

        in_maps.append({"x": xbf, "idxs": wrapped, "wrow": wtile})
    return in_maps


def _run(inputs):
    global _cached
    if _cached is None:
        _cached = _build()
    nc = _cached
    in_maps = _host_prep(inputs["x"], inputs["weight"], inputs["idx"])
    res = run_bass_kernel_spmd(nc, in_maps, list(range(NCORES)))
    parts = [np.asarray(res.results[m]["out"]) for m in range(NCORES)]
    full = np.concatenate(parts, axis=1)  # (NCELL, NJ) bf16
    return full.astype(np.float32).reshape(-1), res


def kernel(**inputs) -> np.ndarray:
    out, _ = _run(inputs)
    return out
